# revision 17
# baseline (speedup 1.0000x reference)
"""NNUE evaluation kernel for Trainium2 (8 NeuronCores, data-parallel batch).

reference math:
    wh = clip(white @ W_ft.T, 0, 1)        # [B, 256]
    bh = clip(black @ W_ft.T, 0, 1)        # [B, 256]
    x  = concat(wh, bh)                    # [B, 512]
    x  = relu(x @ W1.T + b1); x = relu(x @ W2.T + b2)
    ev = (x @ W3.T + b3) * stm[:, None]    # [B, 1]

Strategy: shard B=4096 across 8 cores (512 rows each), data-parallel, no
collectives. The whole net runs in fp16 (2e-2 tolerance; measured rel err
1.7e-3), halving the dominant HBM traffic vs fp32: per core ~105 MB
(features + W_ft); fp16 matmul streams 1 cycle/row so PE time is pure
GEMM (1280 matmuls x 216 ns = 277 us) and total ~307-313 us is within a
few percent of the max(PE, HBM-stream) floor (vs 666 us fp32 baseline).

The host packs, per core, the transposed features AND W_ft.T into ONE
DRAM tensor [128, NT=320, 1280] fp16 where each k-row is
[white 512 | black 512 | W_ftT 256]; every transfer lands directly in
matmul layout (moving [k,b] feature tiles + stationary weight tiles), so
there are no on-chip transposes or DVE evacuations. The stream moves as
4-k-row chunks (10 KB/partition descriptors -- the DMA engines' sweet
spot; 20 KB descriptors run ~10% slower per byte and 3 concurrent queues
thrash HBM) alternating across the two HWDGE queues (SP + Activation),
ending with two 2-row chunks so the last chunk's PE work is short. 56 PE
warm-up matmuls into a scratch PSUM bank both ramp the tensor-engine
p-state (0.65 -> 2.4 GHz; a stalled PE drops to 1.2 GHz for ~3 us) and
deliberately delay the first real matmul ~15 us so the DMA stream builds
a ~4-chunk SBUF lead that rides out HBM rate wobble -- without it the
rate-matched pipeline stalls ~1-2 us every few chunks and pays the
p-state penalty each time. Warm-up tiles live in always-open pools so
stream DMAs never serialize behind them via ring-slot reuse. The MLP
weights ride in one packed fp16 DMA up front on the Pool/SWDGE queue;
biases+stm load at stream end; the clip is fused into the PSUM
evacuation and the tiny MLP stays in transposed [feat, batch] layout.

This walrus build rejects instructions with >1 sync wait, so a post-pass
(_split_multi_waits) redistributes Tile-emitted waits onto single-wait
no-ops.
"""

import sys
import types

import numpy as np


def _inject_ntff_hook():
    """Register the axon NTFF profile hook if this image's antenv lacks it."""
    try:
        import antenv.axon_hooks  # noqa: F401
        return
    except ImportError:
        pass
    try:
        import trn_agent_boot.trn_boot as tb
        hook = tb._ntff_profile_via_ctypes("/opt/axon/libaxon_pjrt.so")
    except Exception:
        hook = None
    mod = types.ModuleType("antenv.axon_hooks")
    mod.get_axon_ntff_profile_hook = lambda: hook
    mod.set_axon_ntff_profile_hook = lambda h: None
    sys.modules["antenv.axon_hooks"] = mod


_inject_ntff_hook()

import concourse.bass as bass
import concourse.mybir as mybir
from concourse.tile import TileContext

N_CORES = 8
B = 4096
BS = B // N_CORES          # 512 batch rows per core
IN = 40960                 # feature count (contraction dim)
H = 256                    # hidden per perspective
NT = IN // 128             # 320 k-rows in the packed stream
ROW = 2 * BS + H           # 1280 fp16 per (p, k-row): white | black | W_ftT
# transfer schedule: uniform 4-row chunks (10 KB/partition descriptors =
# the DMA engines' sweet spot; 20 KB runs ~10% slower per byte), with two
# 2-row chunks at the end so the final chunk's PE work is off the tail.
CHUNKS = [4] * 79 + [2] * 2
assert sum(CHUNKS) == NT

F32 = mybir.dt.float32
F16 = mybir.dt.float16


def _split_multi_waits(nc: bass.Bass) -> None:
    """This walrus build rejects instructions carrying more than one sync
    wait. Split any such instruction: emit single-wait no-ops on the same
    engine immediately before it (same engine stream => same semantics)."""
    for f in nc.m.functions:
        for bb in f.blocks:
            new_insts = []
            changed = False
            for inst in bb.instructions:
                si = inst.sync_info
                waits = list(si.on_wait) if si is not None and si.on_wait else []
                if len(waits) > 1:
                    changed = True
                    for i, w in enumerate(waits[:-1]):
                        nop = mybir.InstNoOp(
                            name=f"{inst.name}-sw{i}", ins=[], outs=[]
                        )
                        nop.engine = inst.engine
                        nop.sync_info = mybir.SyncInfo(on_wait=[w], on_update=[])
                        nc.register_instruction(nop)
                        new_insts.append(nop)
                    inst.sync_info = mybir.SyncInfo(
                        on_wait=[waits[-1]],
                        on_update=list(si.on_update) if si.on_update else [],
                    )
                new_insts.append(inst)
            if changed:
                bb.instructions = new_insts


def build_kernel(**_ignored) -> bass.Bass:
    nc = bass.Bass()

    # packed stream: pk[p, t, 0:512]=white, [512:1024]=black,
    # [1024:1280]=W_ftT, where global k index = t*128 + p.
    pk = nc.dram_tensor("packed", [128, NT, ROW], F16, kind="ExternalInput")
    # packed MLP weights: [:, 0:128]=W1Ts, [0:32, 128:160]=W2T,
    # [0:32, 160]=W3T (rows 32:128 of the tail columns are padding).
    mwd = nc.dram_tensor("mlpw", [128, 161], F16, kind="ExternalInput")
    # biases: [:, 0]=b1, [:, 1]=b2, [0, 2]=b3
    bd = nc.dram_tensor("biases", [32, 3], F32, kind="ExternalInput")
    stm = nc.dram_tensor("side_to_move", [1, BS], F32, kind="ExternalInput")
    out = nc.dram_tensor("evaluation", [1, BS], F32, kind="ExternalOutput")

    with TileContext(nc) as tc:
        with (
            tc.tile_pool(name="ot_psum", bufs=1, space="PSUM") as ot_pool,
            tc.tile_pool(name="warm_psum", bufs=1, space="PSUM") as wpp,
            tc.tile_pool(name="mlp", bufs=1) as mlp,
        ):
            # out.T accumulators: [h-tile 128, b 512] x (2 sides x 2 h-tiles)
            ot = [
                ot_pool.tile([128, BS], F32, tag=f"ot{i}", name=f"ot{i}")
                for i in range(4)
            ]

            # ---- PE warm-up: a few matmuls on a zeroed scratch tile keep
            # the tensor engine p-state ramping toward 2.4 GHz while the
            # first stream chunks arrive; the PSUM bank is never read. These
            # tiles live in always-open pools so the stream DMAs don't
            # serialize behind them via ring-slot reuse. ----
            wsb = mlp.tile([128, BS], F16, tag="warm_sb")
            nc.gpsimd.memset(wsb[:], 0.0)
            # The warm-ups also intentionally DELAY the first real matmul
            # (program order on the PE queue): the DMA stream builds a
            # ~4-chunk lead that rides out run-to-run HBM rate wobble
            # (8 cores share the device HBM at its limit), so the PE never
            # stalls mid-stream (and never drops p-state).
            wps = wpp.tile([128, BS], F32, tag="warm")
            for i in range(56):
                nc.tensor.matmul(
                    wps, wsb[:, :128], wsb[:], start=True, stop=True,
                )

            # MLP weights: one packed fp16 DMA, first thing on the Pool queue
            mw = mlp.tile([128, 161], F16, tag="mlpw")
            nc.gpsimd.dma_start(out=mw[:], in_=mwd[:, :])

            # ---- main loop: feature-transformer GEMMs over the packed
            # stream, chunks alternating across the two HWDGE queues ----
            with tc.tile_pool(name="qslab", bufs=12) as qpool:
                kt_base = 0
                for ci, nk in enumerate(CHUNKS):
                    t = qpool.tile([128, nk, ROW], F16,
                                   tag=f"q{nk}", name=f"c{ci}")
                    q = [nc.sync, nc.scalar][ci % 2]
                    q.dma_start(out=t[:], in_=pk[:, kt_base:kt_base + nk, :])
                    for kt in range(nk):
                        for side in range(2):
                            for h in range(2):
                                nc.tensor.matmul(
                                    ot[side * 2 + h],
                                    t[:, kt,
                                      2 * BS + h * 128:2 * BS + (h + 1) * 128],
                                    t[:, kt, side * BS:(side + 1) * BS],
                                    start=kt_base + kt == 0,
                                    stop=kt_base + kt == NT - 1,
                                )
                    kt_base += nk

            # biases + stm: tiny loads at stream end (needed only in the tail)
            bt = mlp.tile([32, 3], F32, tag="bt")
            nc.scalar.dma_start(out=bt[:], in_=bd[:, :])
            stm_sb = mlp.tile([1, BS], F32, tag="stm")
            nc.scalar.dma_start(out=stm_sb[:], in_=stm[:, :])

            # ---- clip + MLP (transposed layout throughout) ----
            with tc.tile_pool(name="mlp2_psum", bufs=1, space="PSUM") as mpp2:
                xt = []
                for i in range(4):
                    t = mlp.tile([128, BS], F16, tag=f"xt{i}")
                    nc.vector.tensor_scalar(
                        out=t[:], in0=ot[i][:], scalar1=0.0, scalar2=1.0,
                        op0=mybir.AluOpType.max, op1=mybir.AluOpType.min,
                    )
                    xt.append(t)

                h1p = mpp2.tile([32, BS], F32, tag="h1")
                for kt in range(4):
                    nc.tensor.matmul(
                        h1p, mw[:, kt * 32:(kt + 1) * 32], xt[kt][:],
                        start=kt == 0, stop=kt == 3,
                    )
                h1 = mlp.tile([32, BS], F16)
                nc.vector.tensor_scalar(
                    out=h1[:], in0=h1p[:], scalar1=bt[:, 0:1], scalar2=0.0,
                    op0=mybir.AluOpType.add, op1=mybir.AluOpType.max,
                )

                h2p = mpp2.tile([32, BS], F32, tag="h2")
                nc.tensor.matmul(
                    h2p, mw[0:32, 128:160], h1[:], start=True, stop=True
                )
                h2 = mlp.tile([32, BS], F16)
                nc.vector.tensor_scalar(
                    out=h2[:], in0=h2p[:], scalar1=bt[:, 1:2], scalar2=0.0,
                    op0=mybir.AluOpType.add, op1=mybir.AluOpType.max,
                )

                evp = mpp2.tile([1, BS], F32, tag="ev")
                nc.tensor.matmul(
                    evp, mw[0:32, 160:161], h2[:], start=True, stop=True
                )
                ev = mlp.tile([1, BS], F32)
                nc.vector.tensor_scalar(
                    out=ev[:], in0=evp[:], scalar1=bt[0:1, 2:3], scalar2=None,
                    op0=mybir.AluOpType.add,
                )
                evs = mlp.tile([1, BS], F32)
                nc.vector.tensor_mul(out=evs[:], in0=ev[:], in1=stm_sb[:])
                nc.sync.dma_start(out=out[:, :], in_=evs[:])

    _split_multi_waits(nc)
    return nc


_NC_CACHE: dict = {}


def _get_nc(**kwargs) -> bass.Bass:
    key = tuple(sorted(kwargs.items()))
    if key not in _NC_CACHE:
        _NC_CACHE[key] = build_kernel(**kwargs)
    return _NC_CACHE[key]


def make_in_maps(inputs: dict) -> list:
    """Shard full inputs into per-core input maps."""
    wf16 = np.asarray(inputs["white_features"], dtype=np.float32).astype(np.float16)
    bf16 = np.asarray(inputs["black_features"], dtype=np.float32).astype(np.float16)
    w_ftT = np.asarray(inputs["W_ft"], dtype=np.float32).T  # [IN, H]
    # W_ft part of the packed row, same for every core:
    # [128, NT, H] with [p, t, :] = W_ftT[t*128 + p, :]
    w_sw = np.ascontiguousarray(
        w_ftT.reshape(NT, 128, H).transpose(1, 0, 2)
    ).astype(np.float16)
    stm = np.ascontiguousarray(inputs["side_to_move"], dtype=np.float32)
    w1T = np.asarray(inputs["W1"], dtype=np.float32).T  # [512, 32]
    w1Ts = np.ascontiguousarray(
        w1T.reshape(4, 128, 32).transpose(1, 0, 2)
    ).reshape(128, 128).astype(np.float16)
    mlpw = np.zeros((128, 161), dtype=np.float16)
    mlpw[:, 0:128] = w1Ts
    mlpw[0:32, 128:160] = np.asarray(inputs["W2"], dtype=np.float32).T
    mlpw[0:32, 160] = np.asarray(inputs["W3"], dtype=np.float32).reshape(32)
    biases = np.zeros((32, 3), dtype=np.float32)
    biases[:, 0] = np.asarray(inputs["b1"], dtype=np.float32)
    biases[:, 1] = np.asarray(inputs["b2"], dtype=np.float32)
    biases[0, 2] = float(np.asarray(inputs["b3"]).reshape(()))

    maps = []
    for c in range(N_CORES):
        sl = slice(c * BS, (c + 1) * BS)
        pk = np.empty((128, NT, ROW), dtype=np.float16)
        # feats[b, t*128 + p] -> pk[p, t, b]
        pk[:, :, 0:BS] = wf16[sl].reshape(BS, NT, 128).transpose(2, 1, 0)
        pk[:, :, BS:2 * BS] = bf16[sl].reshape(BS, NT, 128).transpose(2, 1, 0)
        pk[:, :, 2 * BS:] = w_sw
        maps.append({
            "packed": pk,
            "mlpw": mlpw,
            "biases": biases,
            "side_to_move": stm[sl].reshape(1, BS),
        })
    return maps


def run(inputs: dict, trace: bool = False, **_ignored):
    """Run on all 8 cores; returns (full_output [4096,1] fp32, BassKernelResults)."""
    from concourse.bass_utils import run_bass_kernel_spmd

    nc = _get_nc()
    res = run_bass_kernel_spmd(
        nc, make_in_maps(inputs), core_ids=list(range(N_CORES)), trace=trace
    )
    full = np.concatenate(
        [res.results[c]["evaluation"].reshape(BS, 1) for c in range(N_CORES)],
        axis=0,
    ).astype(np.float32)
    return full, res


def kernel(**inputs) -> np.ndarray:
    return run(inputs, trace=False)[0]


if __name__ == "__main__":
    rng = np.random.default_rng(0)
    ins = {
        "white_features": rng.random((B, IN), dtype=np.float32),
        "black_features": rng.random((B, IN), dtype=np.float32),
        "side_to_move": np.ones((B,), dtype=np.float32),
        "W_ft": (0.1 * rng.standard_normal((H, IN))).astype(np.float32),
        "W1": (0.06 * rng.standard_normal((32, 2 * H))).astype(np.float32),
        "b1": np.zeros(32, np.float32),
        "W2": (0.17 * rng.standard_normal((32, 32))).astype(np.float32),
        "b2": np.zeros(32, np.float32),
        "W3": (0.24 * rng.standard_normal((1, 32))).astype(np.float32),
        "b3": np.zeros(1, np.float32),
    }
    out = kernel(**ins)
    # host reference
    whr = np.clip(ins["white_features"] @ ins["W_ft"].T, 0, 1)
    bhr = np.clip(ins["black_features"] @ ins["W_ft"].T, 0, 1)
    x = np.concatenate([whr, bhr], axis=1)
    x = np.maximum(x @ ins["W1"].T + ins["b1"], 0)
    x = np.maximum(x @ ins["W2"].T + ins["b2"], 0)
    ref = (x @ ins["W3"].T + ins["b3"]) * ins["side_to_move"][:, None]
    rel = np.linalg.norm(out - ref) / np.linalg.norm(ref)
    print("rel err:", rel)


# revision 18
# speedup vs baseline: 1.0079x; 1.0079x over previous
"""NNUE evaluation kernel for Trainium2 (8 NeuronCores, data-parallel batch).

reference math:
    wh = clip(white @ W_ft.T, 0, 1)        # [B, 256]
    bh = clip(black @ W_ft.T, 0, 1)        # [B, 256]
    x  = concat(wh, bh)                    # [B, 512]
    x  = relu(x @ W1.T + b1); x = relu(x @ W2.T + b2)
    ev = (x @ W3.T + b3) * stm[:, None]    # [B, 1]

Strategy: shard B=4096 across 8 cores (512 rows each), data-parallel, no
collectives. The whole net runs in fp16 (2e-2 tolerance; measured rel err
1.7e-3), halving the dominant HBM traffic vs fp32: per core ~105 MB
(features + W_ft); fp16 matmul streams 1 cycle/row so PE time is pure
GEMM (1280 matmuls x 216 ns = 277 us) and total ~307-313 us is within a
few percent of the max(PE, HBM-stream) floor (vs 666 us fp32 baseline).

The host packs, per core, the transposed features AND W_ft.T into ONE
DRAM tensor [128, NT=320, 1280] fp16 where each k-row is
[white 512 | black 512 | W_ftT 256]; every transfer lands directly in
matmul layout (moving [k,b] feature tiles + stationary weight tiles), so
there are no on-chip transposes or DVE evacuations. The stream moves as
4-k-row chunks (10 KB/partition descriptors -- the DMA engines' sweet
spot; 20 KB descriptors run ~10% slower per byte and 3 concurrent queues
thrash HBM) alternating across the two HWDGE queues (SP + Activation),
ending with two 2-row chunks so the last chunk's PE work is short. 56 PE
warm-up matmuls into a scratch PSUM bank both ramp the tensor-engine
p-state (0.65 -> 2.4 GHz; a stalled PE drops to 1.2 GHz for ~3 us) and
deliberately delay the first real matmul ~15 us so the DMA stream builds
a ~4-chunk SBUF lead that rides out HBM rate wobble -- without it the
rate-matched pipeline stalls ~1-2 us every few chunks and pays the
p-state penalty each time. Warm-up tiles live in always-open pools so
stream DMAs never serialize behind them via ring-slot reuse. The MLP
weights ride in one packed fp16 DMA up front on the Pool/SWDGE queue;
biases+stm load at stream end; the clip is fused into the PSUM
evacuation and the tiny MLP stays in transposed [feat, batch] layout.

This walrus build rejects instructions with >1 sync wait, so a post-pass
(_split_multi_waits) redistributes Tile-emitted waits onto single-wait
no-ops.
"""

import sys
import types

import numpy as np


def _inject_ntff_hook():
    """Register the axon NTFF profile hook if this image's antenv lacks it."""
    try:
        import antenv.axon_hooks  # noqa: F401
        return
    except ImportError:
        pass
    try:
        import trn_agent_boot.trn_boot as tb
        hook = tb._ntff_profile_via_ctypes("/opt/axon/libaxon_pjrt.so")
    except Exception:
        hook = None
    mod = types.ModuleType("antenv.axon_hooks")
    mod.get_axon_ntff_profile_hook = lambda: hook
    mod.set_axon_ntff_profile_hook = lambda h: None
    sys.modules["antenv.axon_hooks"] = mod


_inject_ntff_hook()

import concourse.bass as bass
import concourse.mybir as mybir
from concourse.tile import TileContext

N_CORES = 8
B = 4096
BS = B // N_CORES          # 512 batch rows per core
IN = 40960                 # feature count (contraction dim)
H = 256                    # hidden per perspective
NT = IN // 128             # 320 k-rows in the packed stream
ROW = 2 * BS + H           # 1280 fp16 per (p, k-row): white | black | W_ftT
# transfer schedule: uniform 4-row chunks (10 KB/partition descriptors =
# the DMA engines' sweet spot; 20 KB runs ~10% slower per byte), with two
# 2-row chunks at the end so the final chunk's PE work is off the tail.
CHUNKS = [4] * 79 + [2] * 2
assert sum(CHUNKS) == NT

F32 = mybir.dt.float32
F16 = mybir.dt.float16


def _split_multi_waits(nc: bass.Bass) -> None:
    """This walrus build rejects instructions carrying more than one sync
    wait. Split any such instruction: emit single-wait no-ops on the same
    engine immediately before it (same engine stream => same semantics)."""
    for f in nc.m.functions:
        for bb in f.blocks:
            new_insts = []
            changed = False
            for inst in bb.instructions:
                si = inst.sync_info
                waits = list(si.on_wait) if si is not None and si.on_wait else []
                if len(waits) > 1:
                    changed = True
                    for i, w in enumerate(waits[:-1]):
                        nop = mybir.InstNoOp(
                            name=f"{inst.name}-sw{i}", ins=[], outs=[]
                        )
                        nop.engine = inst.engine
                        nop.sync_info = mybir.SyncInfo(on_wait=[w], on_update=[])
                        nc.register_instruction(nop)
                        new_insts.append(nop)
                    inst.sync_info = mybir.SyncInfo(
                        on_wait=[waits[-1]],
                        on_update=list(si.on_update) if si.on_update else [],
                    )
                new_insts.append(inst)
            if changed:
                bb.instructions = new_insts


def build_kernel(**_ignored) -> bass.Bass:
    nc = bass.Bass()

    # packed stream: pk[p, t, 0:512]=white, [512:1024]=black,
    # [1024:1280]=W_ftT, where global k index = t*128 + p.
    pk = nc.dram_tensor("packed", [128, NT, ROW], F16, kind="ExternalInput")
    # packed MLP weights: [:, 0:128]=W1Ts, [0:32, 128:160]=W2T,
    # [0:32, 160]=W3T (rows 32:128 of the tail columns are padding).
    mwd = nc.dram_tensor("mlpw", [128, 161], F16, kind="ExternalInput")
    # biases: [:, 0]=b1, [:, 1]=b2, [0, 2]=b3
    bd = nc.dram_tensor("biases", [32, 3], F32, kind="ExternalInput")
    stm = nc.dram_tensor("side_to_move", [1, BS], F32, kind="ExternalInput")
    out = nc.dram_tensor("evaluation", [1, BS], F32, kind="ExternalOutput")

    with TileContext(nc) as tc:
        with (
            tc.tile_pool(name="ot_psum", bufs=1, space="PSUM") as ot_pool,
            tc.tile_pool(name="warm_psum", bufs=1, space="PSUM") as wpp,
            tc.tile_pool(name="mlp", bufs=1) as mlp,
        ):
            # out.T accumulators: [h-tile 128, b 512] x (2 sides x 2 h-tiles)
            ot = [
                ot_pool.tile([128, BS], F32, tag=f"ot{i}", name=f"ot{i}")
                for i in range(4)
            ]

            # ---- PE warm-up: a few matmuls on a zeroed scratch tile keep
            # the tensor engine p-state ramping toward 2.4 GHz while the
            # first stream chunks arrive; the PSUM bank is never read. These
            # tiles live in always-open pools so the stream DMAs don't
            # serialize behind them via ring-slot reuse. ----
            wsb = mlp.tile([128, BS], F16, tag="warm_sb")
            nc.gpsimd.memset(wsb[:], 0.0)
            # The warm-ups also intentionally DELAY the first real matmul
            # (program order on the PE queue): the DMA stream builds a
            # ~5.5-chunk lead that rides out run-to-run HBM rate wobble
            # (8 cores share the device HBM at its limit), so the PE never
            # stalls mid-stream (and never drops p-state).
            wps = wpp.tile([128, BS], F32, tag="warm")
            for i in range(80):
                nc.tensor.matmul(
                    wps, wsb[:, :128], wsb[:], start=True, stop=True,
                )

            # MLP weights: one packed fp16 DMA, first thing on the Pool queue
            mw = mlp.tile([128, 161], F16, tag="mlpw")
            nc.gpsimd.dma_start(out=mw[:], in_=mwd[:, :])

            # ---- main loop: feature-transformer GEMMs over the packed
            # stream, chunks alternating across the two HWDGE queues ----
            with tc.tile_pool(name="qslab", bufs=12) as qpool:
                kt_base = 0
                for ci, nk in enumerate(CHUNKS):
                    t = qpool.tile([128, nk, ROW], F16,
                                   tag=f"q{nk}", name=f"c{ci}")
                    q = [nc.sync, nc.scalar][ci % 2]
                    q.dma_start(out=t[:], in_=pk[:, kt_base:kt_base + nk, :])
                    for kt in range(nk):
                        for side in range(2):
                            for h in range(2):
                                nc.tensor.matmul(
                                    ot[side * 2 + h],
                                    t[:, kt,
                                      2 * BS + h * 128:2 * BS + (h + 1) * 128],
                                    t[:, kt, side * BS:(side + 1) * BS],
                                    start=kt_base + kt == 0,
                                    stop=kt_base + kt == NT - 1,
                                )
                    kt_base += nk

            # biases + stm: tiny loads at stream end (needed only in the tail)
            bt = mlp.tile([32, 3], F32, tag="bt")
            nc.scalar.dma_start(out=bt[:], in_=bd[:, :])
            stm_sb = mlp.tile([1, BS], F32, tag="stm")
            nc.scalar.dma_start(out=stm_sb[:], in_=stm[:, :])

            # ---- clip + MLP (transposed layout throughout) ----
            with tc.tile_pool(name="mlp2_psum", bufs=1, space="PSUM") as mpp2:
                xt = []
                for i in range(4):
                    t = mlp.tile([128, BS], F16, tag=f"xt{i}")
                    nc.vector.tensor_scalar(
                        out=t[:], in0=ot[i][:], scalar1=0.0, scalar2=1.0,
                        op0=mybir.AluOpType.max, op1=mybir.AluOpType.min,
                    )
                    xt.append(t)

                h1p = mpp2.tile([32, BS], F32, tag="h1")
                for kt in range(4):
                    nc.tensor.matmul(
                        h1p, mw[:, kt * 32:(kt + 1) * 32], xt[kt][:],
                        start=kt == 0, stop=kt == 3,
                    )
                h1 = mlp.tile([32, BS], F16)
                nc.vector.tensor_scalar(
                    out=h1[:], in0=h1p[:], scalar1=bt[:, 0:1], scalar2=0.0,
                    op0=mybir.AluOpType.add, op1=mybir.AluOpType.max,
                )

                h2p = mpp2.tile([32, BS], F32, tag="h2")
                nc.tensor.matmul(
                    h2p, mw[0:32, 128:160], h1[:], start=True, stop=True
                )
                h2 = mlp.tile([32, BS], F16)
                nc.vector.tensor_scalar(
                    out=h2[:], in0=h2p[:], scalar1=bt[:, 1:2], scalar2=0.0,
                    op0=mybir.AluOpType.add, op1=mybir.AluOpType.max,
                )

                evp = mpp2.tile([1, BS], F32, tag="ev")
                nc.tensor.matmul(
                    evp, mw[0:32, 160:161], h2[:], start=True, stop=True
                )
                ev = mlp.tile([1, BS], F32)
                nc.vector.tensor_scalar(
                    out=ev[:], in0=evp[:], scalar1=bt[0:1, 2:3], scalar2=None,
                    op0=mybir.AluOpType.add,
                )
                evs = mlp.tile([1, BS], F32)
                nc.vector.tensor_mul(out=evs[:], in0=ev[:], in1=stm_sb[:])
                nc.sync.dma_start(out=out[:, :], in_=evs[:])

    _split_multi_waits(nc)
    return nc


_NC_CACHE: dict = {}


def _get_nc(**kwargs) -> bass.Bass:
    key = tuple(sorted(kwargs.items()))
    if key not in _NC_CACHE:
        _NC_CACHE[key] = build_kernel(**kwargs)
    return _NC_CACHE[key]


def make_in_maps(inputs: dict) -> list:
    """Shard full inputs into per-core input maps."""
    wf16 = np.asarray(inputs["white_features"], dtype=np.float32).astype(np.float16)
    bf16 = np.asarray(inputs["black_features"], dtype=np.float32).astype(np.float16)
    w_ftT = np.asarray(inputs["W_ft"], dtype=np.float32).T  # [IN, H]
    # W_ft part of the packed row, same for every core:
    # [128, NT, H] with [p, t, :] = W_ftT[t*128 + p, :]
    w_sw = np.ascontiguousarray(
        w_ftT.reshape(NT, 128, H).transpose(1, 0, 2)
    ).astype(np.float16)
    stm = np.ascontiguousarray(inputs["side_to_move"], dtype=np.float32)
    w1T = np.asarray(inputs["W1"], dtype=np.float32).T  # [512, 32]
    w1Ts = np.ascontiguousarray(
        w1T.reshape(4, 128, 32).transpose(1, 0, 2)
    ).reshape(128, 128).astype(np.float16)
    mlpw = np.zeros((128, 161), dtype=np.float16)
    mlpw[:, 0:128] = w1Ts
    mlpw[0:32, 128:160] = np.asarray(inputs["W2"], dtype=np.float32).T
    mlpw[0:32, 160] = np.asarray(inputs["W3"], dtype=np.float32).reshape(32)
    biases = np.zeros((32, 3), dtype=np.float32)
    biases[:, 0] = np.asarray(inputs["b1"], dtype=np.float32)
    biases[:, 1] = np.asarray(inputs["b2"], dtype=np.float32)
    biases[0, 2] = float(np.asarray(inputs["b3"]).reshape(()))

    maps = []
    for c in range(N_CORES):
        sl = slice(c * BS, (c + 1) * BS)
        pk = np.empty((128, NT, ROW), dtype=np.float16)
        # feats[b, t*128 + p] -> pk[p, t, b]
        pk[:, :, 0:BS] = wf16[sl].reshape(BS, NT, 128).transpose(2, 1, 0)
        pk[:, :, BS:2 * BS] = bf16[sl].reshape(BS, NT, 128).transpose(2, 1, 0)
        pk[:, :, 2 * BS:] = w_sw
        maps.append({
            "packed": pk,
            "mlpw": mlpw,
            "biases": biases,
            "side_to_move": stm[sl].reshape(1, BS),
        })
    return maps


def run(inputs: dict, trace: bool = False, **_ignored):
    """Run on all 8 cores; returns (full_output [4096,1] fp32, BassKernelResults)."""
    from concourse.bass_utils import run_bass_kernel_spmd

    nc = _get_nc()
    res = run_bass_kernel_spmd(
        nc, make_in_maps(inputs), core_ids=list(range(N_CORES)), trace=trace
    )
    full = np.concatenate(
        [res.results[c]["evaluation"].reshape(BS, 1) for c in range(N_CORES)],
        axis=0,
    ).astype(np.float32)
    return full, res


def kernel(**inputs) -> np.ndarray:
    return run(inputs, trace=False)[0]


if __name__ == "__main__":
    rng = np.random.default_rng(0)
    ins = {
        "white_features": rng.random((B, IN), dtype=np.float32),
        "black_features": rng.random((B, IN), dtype=np.float32),
        "side_to_move": np.ones((B,), dtype=np.float32),
        "W_ft": (0.1 * rng.standard_normal((H, IN))).astype(np.float32),
        "W1": (0.06 * rng.standard_normal((32, 2 * H))).astype(np.float32),
        "b1": np.zeros(32, np.float32),
        "W2": (0.17 * rng.standard_normal((32, 32))).astype(np.float32),
        "b2": np.zeros(32, np.float32),
        "W3": (0.24 * rng.standard_normal((1, 32))).astype(np.float32),
        "b3": np.zeros(1, np.float32),
    }
    out = kernel(**ins)
    # host reference
    whr = np.clip(ins["white_features"] @ ins["W_ft"].T, 0, 1)
    bhr = np.clip(ins["black_features"] @ ins["W_ft"].T, 0, 1)
    x = np.concatenate([whr, bhr], axis=1)
    x = np.maximum(x @ ins["W1"].T + ins["b1"], 0)
    x = np.maximum(x @ ins["W2"].T + ins["b2"], 0)
    ref = (x @ ins["W3"].T + ins["b3"]) * ins["side_to_move"][:, None]
    rel = np.linalg.norm(out - ref) / np.linalg.norm(ref)
    print("rel err:", rel)


# revision 19
# speedup vs baseline: 1.1022x; 1.0935x over previous
"""NNUE evaluation kernel for Trainium2 (8 NeuronCores, data-parallel batch).

reference math:
    wh = clip(white @ W_ft.T, 0, 1)        # [B, 256]
    bh = clip(black @ W_ft.T, 0, 1)        # [B, 256]
    x  = concat(wh, bh)                    # [B, 512]
    x  = relu(x @ W1.T + b1); x = relu(x @ W2.T + b2)
    ev = (x @ W3.T + b3) * stm[:, None]    # [B, 1]

Strategy: shard B=4096 across 8 cores (512 rows each), data-parallel, no
collectives. The whole net runs in fp16 (2e-2 tolerance; measured rel err
1.7e-3), halving the dominant HBM traffic vs fp32: per core ~105 MB
(features + W_ft); fp16 matmul streams 1 cycle/row so PE time is pure
GEMM (1280 matmuls x 216 ns = 277 us) and total ~307-313 us is within a
few percent of the max(PE, HBM-stream) floor (vs 666 us fp32 baseline).

The host packs, per core, the transposed features AND W_ft.T into ONE
DRAM tensor [128, NT=320, 1280] fp16 where each k-row is
[white 512 | black 512 | W_ftT 256]; every transfer lands directly in
matmul layout (moving [k,b] feature tiles + stationary weight tiles), so
there are no on-chip transposes or DVE evacuations. The stream moves as
4-k-row chunks (10 KB/partition descriptors -- the DMA engines' sweet
spot; 20 KB descriptors run ~10% slower per byte and 3 concurrent queues
thrash HBM) alternating across the two HWDGE queues (SP + Activation),
ending with two 2-row chunks so the last chunk's PE work is short. 56 PE
warm-up matmuls into a scratch PSUM bank both ramp the tensor-engine
p-state (0.65 -> 2.4 GHz; a stalled PE drops to 1.2 GHz for ~3 us) and
deliberately delay the first real matmul ~15 us so the DMA stream builds
a ~4-chunk SBUF lead that rides out HBM rate wobble -- without it the
rate-matched pipeline stalls ~1-2 us every few chunks and pays the
p-state penalty each time. Warm-up tiles live in always-open pools so
stream DMAs never serialize behind them via ring-slot reuse. The MLP
weights ride in one packed fp16 DMA up front on the Pool/SWDGE queue;
biases+stm load at stream end; the clip is fused into the PSUM
evacuation and the tiny MLP stays in transposed [feat, batch] layout.

This walrus build rejects instructions with >1 sync wait, so a post-pass
(_split_multi_waits) redistributes Tile-emitted waits onto single-wait
no-ops.
"""

import sys
import types

import numpy as np


def _inject_ntff_hook():
    """Register the axon NTFF profile hook if this image's antenv lacks it."""
    try:
        import antenv.axon_hooks  # noqa: F401
        return
    except ImportError:
        pass
    try:
        import trn_agent_boot.trn_boot as tb
        hook = tb._ntff_profile_via_ctypes("/opt/axon/libaxon_pjrt.so")
    except Exception:
        hook = None
    mod = types.ModuleType("antenv.axon_hooks")
    mod.get_axon_ntff_profile_hook = lambda: hook
    mod.set_axon_ntff_profile_hook = lambda h: None
    sys.modules["antenv.axon_hooks"] = mod


_inject_ntff_hook()

import concourse.bass as bass
import concourse.mybir as mybir
from concourse.tile import TileContext

N_CORES = 8
B = 4096
BS = B // N_CORES          # 512 batch rows per core
IN = 40960                 # feature count (contraction dim)
H = 256                    # hidden per perspective
NT = IN // 128             # 320 k-rows in the packed stream
ROW = 2 * BS + H           # 1280 fp16 per (p, k-row): white | black | W_ftT
# transfer schedule: uniform 4-row chunks (10 KB/partition descriptors =
# the DMA engines' sweet spot; 20 KB runs ~10% slower per byte), with two
# 2-row chunks at the end so the final chunk's PE work is off the tail.
CHUNKS = [4] * 79 + [2] * 2
assert sum(CHUNKS) == NT

F32 = mybir.dt.float32
F16 = mybir.dt.float16


def _split_multi_waits(nc: bass.Bass) -> None:
    """This walrus build rejects instructions carrying more than one sync
    wait. Split any such instruction: emit single-wait no-ops on the same
    engine immediately before it (same engine stream => same semantics)."""
    for f in nc.m.functions:
        for bb in f.blocks:
            new_insts = []
            changed = False
            for inst in bb.instructions:
                si = inst.sync_info
                waits = list(si.on_wait) if si is not None and si.on_wait else []
                if len(waits) > 1:
                    changed = True
                    for i, w in enumerate(waits[:-1]):
                        nop = mybir.InstNoOp(
                            name=f"{inst.name}-sw{i}", ins=[], outs=[]
                        )
                        nop.engine = inst.engine
                        nop.sync_info = mybir.SyncInfo(on_wait=[w], on_update=[])
                        nc.register_instruction(nop)
                        new_insts.append(nop)
                    inst.sync_info = mybir.SyncInfo(
                        on_wait=[waits[-1]],
                        on_update=list(si.on_update) if si.on_update else [],
                    )
                new_insts.append(inst)
            if changed:
                bb.instructions = new_insts


def build_kernel(**_ignored) -> bass.Bass:
    nc = bass.Bass()

    # packed stream: pk[p, t, 0:512]=white, [512:1024]=black,
    # [1024:1280]=W_ftT, where global k index = t*128 + p.
    pk = nc.dram_tensor("packed", [128, NT, ROW], F16, kind="ExternalInput")
    # packed MLP weights: [:, 0:128]=W1Ts, [0:32, 128:160]=W2T,
    # [0:32, 160]=W3T (rows 32:128 of the tail columns are padding).
    mwd = nc.dram_tensor("mlpw", [128, 161], F16, kind="ExternalInput")
    # biases: [:, 0]=b1, [:, 1]=b2, [0, 2]=b3
    bd = nc.dram_tensor("biases", [32, 3], F32, kind="ExternalInput")
    stm = nc.dram_tensor("side_to_move", [1, BS], F32, kind="ExternalInput")
    out = nc.dram_tensor("evaluation", [1, BS], F32, kind="ExternalOutput")

    with TileContext(nc) as tc:
        with (
            tc.tile_pool(name="ot_psum", bufs=1, space="PSUM") as ot_pool,
            tc.tile_pool(name="warm_psum", bufs=1, space="PSUM") as wpp,
            tc.tile_pool(name="mlp", bufs=1) as mlp,
        ):
            # out.T accumulators: [h-tile 128, b 512] x (2 sides x 2 h-tiles)
            ot = [
                ot_pool.tile([128, BS], F32, tag=f"ot{i}", name=f"ot{i}")
                for i in range(4)
            ]

            # ---- PE warm-up: a few matmuls on a zeroed scratch tile keep
            # the tensor engine p-state ramping toward 2.4 GHz while the
            # first stream chunks arrive; the PSUM bank is never read. These
            # tiles live in always-open pools so the stream DMAs don't
            # serialize behind them via ring-slot reuse. ----
            wsb = mlp.tile([128, BS], F16, tag="warm_sb")
            nc.gpsimd.memset(wsb[:], 0.0)
            h2 = mlp.tile([33, BS], F16, tag="h2t")
            nc.gpsimd.memset(h2[32:33, :], 1.0)
            # The warm-ups also intentionally DELAY the first real matmul
            # (program order on the PE queue): the DMA stream builds a
            # ~5.5-chunk lead that rides out run-to-run HBM rate wobble
            # (8 cores share the device HBM at its limit), so the PE never
            # stalls mid-stream (and never drops p-state).
            wps = wpp.tile([128, BS], F32, tag="warm")
            for i in range(80):
                nc.tensor.matmul(
                    wps, wsb[:, :128], wsb[:], start=True, stop=True,
                )

            # MLP weights: one packed fp16 DMA, first thing on the Pool queue
            mw = mlp.tile([128, 161], F16, tag="mlpw")
            nc.gpsimd.dma_start(out=mw[:], in_=mwd[:, :])

            # tiny queue-warmer DMAs: pay each HWDGE queue's one-time setup
            # before the first stream chunk so its descriptors flow sooner
            qw = mlp.tile([1, 2], F32, tag="qwarm")
            nc.sync.dma_start(out=qw[:, 0:1], in_=bd[0:1, 0:1])
            nc.scalar.dma_start(out=qw[:, 1:2], in_=bd[0:1, 0:1])

            # ---- main loop: feature-transformer GEMMs over the packed
            # stream, chunks alternating across the two HWDGE queues ----
            with tc.tile_pool(name="qslab", bufs=12) as qpool:
                kt_base = 0
                for ci, nk in enumerate(CHUNKS):
                    t = qpool.tile([128, nk, ROW], F16,
                                   tag=f"q{nk}", name=f"c{ci}")
                    q = [nc.sync, nc.scalar][ci % 2]
                    q.dma_start(out=t[:], in_=pk[:, kt_base:kt_base + nk, :])
                    for kt in range(nk):
                        for side in range(2):
                            for h in range(2):
                                nc.tensor.matmul(
                                    ot[side * 2 + h],
                                    t[:, kt,
                                      2 * BS + h * 128:2 * BS + (h + 1) * 128],
                                    t[:, kt, side * BS:(side + 1) * BS],
                                    start=kt_base + kt == 0,
                                    stop=kt_base + kt == NT - 1,
                                )
                    kt_base += nk

            # biases + stm: tiny loads at stream end (needed only in the tail)
            bt = mlp.tile([32, 3], F32, tag="bt")
            nc.scalar.dma_start(out=bt[:], in_=bd[:, :])
            stm_sb = mlp.tile([1, BS], F32, tag="stm")
            nc.scalar.dma_start(out=stm_sb[:], in_=stm[:, :])

            # ---- clip + MLP (transposed layout throughout) ----
            with tc.tile_pool(name="mlp2_psum", bufs=1, space="PSUM") as mpp2:
                xt = []
                for i in range(4):
                    t = mlp.tile([128, BS], F16, tag=f"xt{i}")
                    nc.vector.tensor_scalar(
                        out=t[:], in0=ot[i][:], scalar1=0.0, scalar2=1.0,
                        op0=mybir.AluOpType.max, op1=mybir.AluOpType.min,
                    )
                    xt.append(t)

                h1p = mpp2.tile([32, BS], F32, tag="h1")
                for kt in range(4):
                    nc.tensor.matmul(
                        h1p, mw[:, kt * 32:(kt + 1) * 32], xt[kt][:],
                        start=kt == 0, stop=kt == 3,
                    )
                h1 = mlp.tile([32, BS], F16)
                nc.vector.tensor_scalar(
                    out=h1[:], in0=h1p[:], scalar1=bt[:, 0:1], scalar2=0.0,
                    op0=mybir.AluOpType.add, op1=mybir.AluOpType.max,
                )

                h2p = mpp2.tile([32, BS], F32, tag="h2")
                nc.tensor.matmul(
                    h2p, mw[0:32, 128:160], h1[:], start=True, stop=True
                )
                nc.vector.tensor_scalar(
                    out=h2[0:32, :], in0=h2p[:], scalar1=bt[:, 1:2],
                    scalar2=0.0,
                    op0=mybir.AluOpType.add, op1=mybir.AluOpType.max,
                )

                # W3' = [W3T; b3] against h2' = [h2; ones] folds the bias
                # into the matmul -- one less DVE hop in the serial tail
                evp = mpp2.tile([1, BS], F32, tag="ev")
                nc.tensor.matmul(
                    evp, mw[0:33, 160:161], h2[:], start=True, stop=True
                )
                evs = mlp.tile([1, BS], F32)
                nc.vector.tensor_mul(out=evs[:], in0=evp[:], in1=stm_sb[:])
                nc.sync.dma_start(out=out[:, :], in_=evs[:])

    _split_multi_waits(nc)
    return nc


_NC_CACHE: dict = {}


def _get_nc(**kwargs) -> bass.Bass:
    key = tuple(sorted(kwargs.items()))
    if key not in _NC_CACHE:
        _NC_CACHE[key] = build_kernel(**kwargs)
    return _NC_CACHE[key]


def make_in_maps(inputs: dict) -> list:
    """Shard full inputs into per-core input maps."""
    wf16 = np.asarray(inputs["white_features"], dtype=np.float32).astype(np.float16)
    bf16 = np.asarray(inputs["black_features"], dtype=np.float32).astype(np.float16)
    w_ftT = np.asarray(inputs["W_ft"], dtype=np.float32).T  # [IN, H]
    # W_ft part of the packed row, same for every core:
    # [128, NT, H] with [p, t, :] = W_ftT[t*128 + p, :]
    w_sw = np.ascontiguousarray(
        w_ftT.reshape(NT, 128, H).transpose(1, 0, 2)
    ).astype(np.float16)
    stm = np.ascontiguousarray(inputs["side_to_move"], dtype=np.float32)
    w1T = np.asarray(inputs["W1"], dtype=np.float32).T  # [512, 32]
    w1Ts = np.ascontiguousarray(
        w1T.reshape(4, 128, 32).transpose(1, 0, 2)
    ).reshape(128, 128).astype(np.float16)
    mlpw = np.zeros((128, 161), dtype=np.float16)
    mlpw[:, 0:128] = w1Ts
    mlpw[0:32, 128:160] = np.asarray(inputs["W2"], dtype=np.float32).T
    mlpw[0:32, 160] = np.asarray(inputs["W3"], dtype=np.float32).reshape(32)
    mlpw[32, 160] = float(np.asarray(inputs["b3"]).reshape(()))
    biases = np.zeros((32, 3), dtype=np.float32)
    biases[:, 0] = np.asarray(inputs["b1"], dtype=np.float32)
    biases[:, 1] = np.asarray(inputs["b2"], dtype=np.float32)
    biases[0, 2] = float(np.asarray(inputs["b3"]).reshape(()))

    maps = []
    for c in range(N_CORES):
        sl = slice(c * BS, (c + 1) * BS)
        pk = np.empty((128, NT, ROW), dtype=np.float16)
        # feats[b, t*128 + p] -> pk[p, t, b]
        pk[:, :, 0:BS] = wf16[sl].reshape(BS, NT, 128).transpose(2, 1, 0)
        pk[:, :, BS:2 * BS] = bf16[sl].reshape(BS, NT, 128).transpose(2, 1, 0)
        pk[:, :, 2 * BS:] = w_sw
        maps.append({
            "packed": pk,
            "mlpw": mlpw,
            "biases": biases,
            "side_to_move": stm[sl].reshape(1, BS),
        })
    return maps


def run(inputs: dict, trace: bool = False, **_ignored):
    """Run on all 8 cores; returns (full_output [4096,1] fp32, BassKernelResults)."""
    from concourse.bass_utils import run_bass_kernel_spmd

    nc = _get_nc()
    res = run_bass_kernel_spmd(
        nc, make_in_maps(inputs), core_ids=list(range(N_CORES)), trace=trace
    )
    full = np.concatenate(
        [res.results[c]["evaluation"].reshape(BS, 1) for c in range(N_CORES)],
        axis=0,
    ).astype(np.float32)
    return full, res


def kernel(**inputs) -> np.ndarray:
    return run(inputs, trace=False)[0]


if __name__ == "__main__":
    rng = np.random.default_rng(0)
    ins = {
        "white_features": rng.random((B, IN), dtype=np.float32),
        "black_features": rng.random((B, IN), dtype=np.float32),
        "side_to_move": np.ones((B,), dtype=np.float32),
        "W_ft": (0.1 * rng.standard_normal((H, IN))).astype(np.float32),
        "W1": (0.06 * rng.standard_normal((32, 2 * H))).astype(np.float32),
        "b1": np.zeros(32, np.float32),
        "W2": (0.17 * rng.standard_normal((32, 32))).astype(np.float32),
        "b2": np.zeros(32, np.float32),
        "W3": (0.24 * rng.standard_normal((1, 32))).astype(np.float32),
        "b3": np.zeros(1, np.float32),
    }
    out = kernel(**ins)
    # host reference
    whr = np.clip(ins["white_features"] @ ins["W_ft"].T, 0, 1)
    bhr = np.clip(ins["black_features"] @ ins["W_ft"].T, 0, 1)
    x = np.concatenate([whr, bhr], axis=1)
    x = np.maximum(x @ ins["W1"].T + ins["b1"], 0)
    x = np.maximum(x @ ins["W2"].T + ins["b2"], 0)
    ref = (x @ ins["W3"].T + ins["b3"]) * ins["side_to_move"][:, None]
    rel = np.linalg.norm(out - ref) / np.linalg.norm(ref)
    print("rel err:", rel)
